# revision 1
# baseline (speedup 1.0000x reference)
"""DirGATConv (2-direction GAT layer blend) on 8 Trainium2 NeuronCores.

Strategy (per direction, per core):
  - Core k owns dst-node range [6250k, 6250(k+1)) for BOTH edge directions;
    outputs are disjoint so no cross-core collectives are needed.
  - Softmax decoupling: att = w_e / sum_seg(w_e) with w_e = exp(lrelu(
    e_src[src]+e_dst[dst])); the segment-max cancels between numerator and
    denominator and value ranges make overflow impossible, so each edge's
    weight is local.
  - Edges laid out CSR column-major per 128-dst-node window (slot s, node d
    on partition d).  Aggregation = PSUM-accumulated identity-matmuls over
    slots (PE performs the segment sum); e_dst is a plain per-window DMA.
  - Per-edge h rows + e_src gathered from a device-built table
    [h bf16 x128 | e_src f32 x4] (512B rows) via the SWDGE dma_gather
    ucode.  int16 index range is handled by a lo/hi table-base split with
    sentinel rows (sentinel e_src = -1e30 => w=0) used for padding slots.
  - Normalized rows are scattered with dma_scatter_add into per-direction
    accumulators; a final pass blends (alpha folded into a x2 denominator
    scale) and adds the bias blend.

Host does integer index preprocessing only (sorting, CSR packing, int16
encoding); all floating-point model compute runs on the NeuronCores.
"""

import numpy as np

N = 50000
F_IN = 128
C_OUT = 32
H = 4
FEAT = H * C_OUT  # 128
ALPHA = 0.5
SLOPE = 0.2
NCORES = 8
NPC = N // NCORES            # nodes per core (6250)
WIN = 128                    # dst nodes per window
NW = (NPC + WIN - 1) // WIN  # windows per core (49)
NPC_PAD = NW * WIN           # 6272
LO_MAX = 32766               # src <= LO_MAX goes to the lo gather call
HI_BASE = 17234              # hi call gathers table rows [HI_BASE, HI_BASE+32768)
TROWS = N + 2                # table rows: [sent | nodes 0..N-1 | sent]
TCOLS = 256                  # bf16 cols: [h x128 | es f32 as 4x2 | dead]
MAX_CHUNK_SLOTS = 8          # max slots per dma_gather call (desc-ring cap ~1024)
TRASH = NPC                  # scatter row for dummy/padding window rows
PHASES = 4                   # debug: 1=const only, 2=+tables, 3=+gather/agg, 4=all
DMA_SCRATCH = 16384          # SWDGE descriptor-ring carveout bytes


# ---------------------------------------------------------------------------
# Host-side graph preprocessing (integer metadata only)
# ---------------------------------------------------------------------------

def _wrap16(vals):
    """Per-call idx wrapping: idx j -> (partition j%16, col j//16), replicated
    to 128 partitions.  vals length must be a multiple of 16."""
    a = np.asarray(vals, dtype=np.int16).reshape(-1, 16).T  # [16, L/16]
    return np.tile(a, (8, 1))  # [128, L/16]


def _prep_direction(src, dst):
    """Per-core CSR structures for one direction.

    Returns (per_core list of dicts, harmonized KLO[w], KHI[w] lists).
    """
    order = np.argsort(dst, kind="stable")
    src_s = src[order]
    dst_s = dst[order]
    deg = np.bincount(dst_s, minlength=N)
    row_start = np.zeros(N + 1, dtype=np.int64)
    np.cumsum(deg, out=row_start[1:])

    cores = []
    for k in range(NCORES):
        n0 = k * NPC
        e0, e1 = row_start[n0], row_start[n0 + NPC]
        s_k = src_s[e0:e1]
        d_loc = dst_s[e0:e1] - n0
        lo_mask = s_k <= LO_MAX
        lo_deg = np.bincount(d_loc[lo_mask], minlength=NPC)
        hi_deg = np.bincount(d_loc[~lo_mask], minlength=NPC)
        perm = np.lexsort((hi_deg, lo_deg))  # by lo_deg, then hi_deg
        cores.append({
            "src": s_k, "d_loc": d_loc, "lo_mask": lo_mask,
            "lo_deg": lo_deg, "hi_deg": hi_deg, "perm": perm,
            "start": row_start[n0:n0 + NPC + 1] - e0,
        })

    klo = np.zeros(NW, dtype=np.int64)
    khi = np.zeros(NW, dtype=np.int64)
    for c in cores:
        ld = c["lo_deg"][c["perm"]]
        hd = c["hi_deg"][c["perm"]]
        ld = np.concatenate([ld, np.zeros(NPC_PAD - NPC, np.int64)])
        hd = np.concatenate([hd, np.zeros(NPC_PAD - NPC, np.int64)])
        klo = np.maximum(klo, ld.reshape(NW, WIN).max(1))
        khi = np.maximum(khi, hd.reshape(NW, WIN).max(1))
    return cores, klo.tolist(), khi.tolist()


def _pack_core_direction(c, klo, khi):
    """Build the int16 gather-idx stream, scatter-idx stream, and perm for one
    (core, direction).  Gather stream layout per window: lo grid column-major
    [slot, 128 nodes], then hi grid; wrapped per MAX_CHUNK_SLOTS chunks."""
    perm = c["perm"]
    start = c["start"]
    src = c["src"]
    lo_mask = c["lo_mask"]
    lo_deg = c["lo_deg"]
    hi_deg = c["hi_deg"]

    # per-local-node edge lists, split lo/hi (dst-sorted so contiguous)
    # lo_srcs[i] = sorted positions; vectorized ragged extraction
    lo_srcs = [None] * NPC
    hi_srcs = [None] * NPC
    for i in range(NPC):
        seg = src[start[i]:start[i + 1]]
        m = lo_mask[start[i]:start[i + 1]]
        lo_srcs[i] = seg[m]
        hi_srcs[i] = seg[~m]

    gcols = []
    sidx_cols = []
    for w in range(NW):
        rows = perm[w * WIN:(w + 1) * WIN]
        nrows = len(rows)
        KL, KH = klo[w], khi[w]
        # lo grid [KL, 128] int16: sentinel 0, real src+1
        # hi grid sentinel points at table row N+1 (second sentinel row)
        lo_g = np.zeros((KL, WIN), dtype=np.int16)
        hi_g = np.full((KH, WIN), N + 1 - HI_BASE, dtype=np.int16)
        for p in range(nrows):
            i = rows[p]
            ls = lo_srcs[i]
            if len(ls):
                lo_g[: len(ls), p] = (ls + 1).astype(np.int16)
            hs = hi_srcs[i]
            if len(hs):
                hi_g[: len(hs), p] = (hs - (HI_BASE - 1)).astype(np.int16)
        grid = np.concatenate([lo_g, hi_g], 0).reshape(-1)  # [(KL+KH)*128]
        # wrap per chunk of MAX_CHUNK_SLOTS slots
        nblk = KL + KH
        for s0 in range(0, nblk, MAX_CHUNK_SLOTS):
            s1 = min(s0 + MAX_CHUNK_SLOTS, nblk)
            gcols.append(_wrap16(grid[s0 * WIN:s1 * WIN]))
        srow = np.full(WIN, TRASH, dtype=np.int16)
        srow[:nrows] = rows.astype(np.int16)
        sidx_cols.append(_wrap16(srow))

    gidx = np.concatenate(gcols, axis=1) if gcols else np.zeros((128, 0), np.int16)
    sidx = np.concatenate(sidx_cols, axis=1)
    return gidx, sidx


def _xperm(x, k, perm):
    xp = np.zeros((NPC_PAD, F_IN), dtype=np.float32)
    xp[:NPC] = x[k * NPC + perm]
    return xp


def _blockdiag(a_vec):
    """[H, C_OUT] -> [FEAT, H] block diagonal placement (no arithmetic)."""
    bd = np.zeros((FEAT, H), dtype=np.float32)
    for h in range(H):
        bd[h * C_OUT:(h + 1) * C_OUT, h] = a_vec[h]
    return bd


def host_prep(x, edge_index, a_src1, a_dst1, a_src2, a_dst2, b1, b2):
    x = np.asarray(x, dtype=np.float32)
    ei = np.asarray(edge_index)
    loops = np.arange(N, dtype=np.int64)
    src_f = np.concatenate([ei[0], loops])
    dst_f = np.concatenate([ei[1], loops])
    src_b = np.concatenate([ei[1], loops])
    dst_b = np.concatenate([ei[0], loops])

    cores_f, klo_f, khi_f = _prep_direction(src_f, dst_f)
    cores_b, klo_b, khi_b = _prep_direction(src_b, dst_b)

    bd1 = np.concatenate([_blockdiag(np.asarray(a_src1, np.float32)),
                          _blockdiag(np.asarray(a_dst1, np.float32))], axis=1)
    bd2 = np.concatenate([_blockdiag(np.asarray(a_src2, np.float32)),
                          _blockdiag(np.asarray(a_dst2, np.float32))], axis=1)
    b1t = np.tile(np.asarray(b1, np.float32)[None, :], (128, 1))
    b2t = np.tile(np.asarray(b2, np.float32)[None, :], (128, 1))

    in_maps = []
    for k in range(NCORES):
        g1, s1 = _pack_core_direction(cores_f[k], klo_f, khi_f)
        g2, s2 = _pack_core_direction(cores_b[k], klo_b, khi_b)
        in_maps.append({
            "x": x,
            "xp1": _xperm(x, k, cores_f[k]["perm"]),
            "xp2": _xperm(x, k, cores_b[k]["perm"]),
            "gidx1": g1, "gidx2": g2,
            "sidx1": s1, "sidx2": s2,
            "bd1": bd1, "bd2": bd2,
            "b1t": b1t, "b2t": b2t,
        })
    struct = {"klo": [klo_f, klo_b], "khi": [khi_f, khi_b]}
    return in_maps, struct


# ---------------------------------------------------------------------------
# Device program
# ---------------------------------------------------------------------------

def build_program(struct):
    import concourse.bass as bass
    import concourse.mybir as mybir
    import concourse.tile as tile
    from concourse.masks import make_identity
    from concourse.library_config import mlp
    from contextlib import ExitStack

    f32 = mybir.dt.float32
    bf16 = mybir.dt.bfloat16
    i16 = mybir.dt.int16

    klo = struct["klo"]
    khi = struct["khi"]
    NBMAX = max(klo[d][w] + khi[d][w] for d in range(2) for w in range(NW))
    gidx_cols = [sum((klo[d][w] + khi[d][w]) * 8 for w in range(NW)) for d in range(2)]

    nc = bass.Bass(num_swdge_queues=4, dynamic_dma_scratch_size=DMA_SCRATCH)
    x_in = nc.dram_tensor("x", [N, F_IN], f32, kind="ExternalInput")
    xp = [nc.dram_tensor(f"xp{d+1}", [NPC_PAD, F_IN], f32, kind="ExternalInput")
          for d in range(2)]
    W = [nc.dram_tensor(f"W{d+1}", [F_IN, FEAT], f32, kind="ExternalInput")
         for d in range(2)]
    bd = [nc.dram_tensor(f"bd{d+1}", [FEAT, 2 * H], f32, kind="ExternalInput")
          for d in range(2)]
    bt = [nc.dram_tensor(f"b{d+1}t", [128, FEAT], f32, kind="ExternalInput")
          for d in range(2)]
    gidx = [nc.dram_tensor(f"gidx{d+1}", [128, gidx_cols[d]], i16, kind="ExternalInput")
            for d in range(2)]
    sidx = [nc.dram_tensor(f"sidx{d+1}", [128, NW * 8], i16, kind="ExternalInput")
            for d in range(2)]

    tab = [nc.dram_tensor(f"tab{d+1}", [TROWS, TCOLS], bf16, kind="Internal")
           for d in range(2)]
    edp = [nc.dram_tensor(f"edp{d+1}", [NPC_PAD, H], f32, kind="Internal")
           for d in range(2)]
    acc = [nc.dram_tensor(f"acc{d+1}", [NPC_PAD, FEAT], f32, kind="Internal")
           for d in range(2)]
    out_ext = nc.dram_tensor("out", [NPC, FEAT], f32, kind="ExternalOutput")

    NT = (N + 127) // 128  # x tiles (last partial)

    with ExitStack() as ctx:
        tc = ctx.enter_context(tile.TileContext(nc))
        const = ctx.enter_context(tc.tile_pool(name="const", bufs=1))
        sb = ctx.enter_context(tc.tile_pool(name="sb", bufs=3))
        sb_g = ctx.enter_context(tc.tile_pool(name="sbg", bufs=2))
        ps_big = ctx.enter_context(tc.tile_pool(name="psb", bufs=3, space="PSUM"))
        ps_acc = ctx.enter_context(tc.tile_pool(name="psa", bufs=3, space="PSUM"))
        ps_sm = ctx.enter_context(tc.tile_pool(name="pss", bufs=2, space="PSUM"))

        # ---- constants ----
        nc.gpsimd.load_library(mlp)
        _regs = {}

        def nreg(v):
            if v not in _regs:
                _regs[v] = nc.gpsimd.to_reg(v)
            return _regs[v]
        id_f32 = const.tile([128, 128], f32)
        make_identity(nc, id_f32[:])
        id_bf = const.tile([128, 128], bf16)
        nc.vector.tensor_copy(out=id_bf[:], in_=id_f32[:])

        wa_sb = []
        w_sbs = []
        for d in range(2):
            w_sb = const.tile([F_IN, FEAT], f32, tag=f"w_sb{d}")
            nc.sync.dma_start(out=w_sb[:], in_=W[d][:, :])
            w_sbs.append(w_sb)
            wt_ps = ps_big.tile([128, 128], f32, tag="psb")
            nc.tensor.transpose(out=wt_ps[:], in_=w_sb[:], identity=id_f32[:])
            wt_sb = sb.tile([128, 128], f32, tag="wt_sb")
            nc.vector.tensor_copy(out=wt_sb[:], in_=wt_ps[:])
            bd_sb = sb.tile([FEAT, 2 * H], f32, tag="bd_sb")
            nc.sync.dma_start(out=bd_sb[:], in_=bd[d][:, :])
            wa_ps = ps_sm.tile([128, 2 * H], f32, tag="pss")
            nc.tensor.matmul(out=wa_ps[:], lhsT=wt_sb[:], rhs=bd_sb[:],
                             start=True, stop=True)
            wa = const.tile([128, 2 * H], f32, tag=f"wa{d}")
            nc.vector.tensor_copy(out=wa[:], in_=wa_ps[:])
            wa_sb.append(wa)

        # bias blend (1-a)*b1 + a*b2, broadcast to 128 partitions host-side
        bbar = const.tile([128, FEAT], f32)
        t_b1 = sb.tile([128, FEAT], f32, tag="tb")
        nc.sync.dma_start(out=t_b1[:], in_=bt[0][:, :])
        t_b2 = sb.tile([128, FEAT], f32, tag="tb2")
        nc.sync.dma_start(out=t_b2[:], in_=bt[1][:, :])
        nc.vector.tensor_scalar(out=t_b1[:], in0=t_b1[:], scalar1=1.0 - ALPHA,
                                scalar2=None, op0=mybir.AluOpType.mult)
        nc.vector.tensor_scalar(out=t_b2[:], in0=t_b2[:], scalar1=ALPHA,
                                scalar2=None, op0=mybir.AluOpType.mult)
        nc.vector.tensor_add(out=bbar[:], in0=t_b1[:], in1=t_b2[:])

        # sentinel rows: h=0, es=-1e30
        sent = const.tile([1, 136], bf16)
        nc.gpsimd.memset(sent[:, 0:128], 0.0)
        nc.gpsimd.memset(sent[:, 128:136].bitcast(f32), -1e30)
        for d in range(2):
            nc.sync.dma_start(out=tab[d][0:1, 0:136], in_=sent[:])
            nc.sync.dma_start(out=tab[d][N + 1:N + 2, 0:136], in_=sent[:])

        # zero the scatter accumulators (Internal DRAM is uninitialized)
        zt = const.tile([128, FEAT], f32)
        nc.gpsimd.memset(zt[:], 0.0)
        for d in range(2):
            for t in range(NW):
                nc.sync.dma_start(out=acc[d][t * 128:(t + 1) * 128, :], in_=zt[:])

        # ---- phase 1: tables (h | es) for all N nodes, both directions ----
        for t in range(NT if PHASES >= 2 else 0):
            r0 = t * 128
            rc = min(128, N - r0)
            xt = sb.tile([128, F_IN], f32, tag="xt")
            nc.sync.dma_start(out=xt[:rc, :], in_=x_in[r0:r0 + rc, :])
            xT_ps = ps_big.tile([128, 128], f32, tag="psb")
            nc.tensor.transpose(out=xT_ps[:, :rc], in_=xt[:rc, :],
                                identity=id_f32[:rc, :rc])
            xT = sb.tile([128, 128], f32, tag="xT")
            nc.vector.tensor_copy(out=xT[:, :rc], in_=xT_ps[:, :rc])
            for d in range(2):
                h_ps = ps_big.tile([128, FEAT], f32, tag="psb")
                nc.tensor.matmul(out=h_ps[:rc, :], lhsT=xT[:, :rc],
                                 rhs=w_sbs[d][:], start=True, stop=True)
                es_ps = ps_sm.tile([128, H], f32, tag="pss")
                nc.tensor.matmul(out=es_ps[:rc, :], lhsT=xT[:, :rc],
                                 rhs=wa_sb[d][:, 0:H], start=True, stop=True)
                stg = sb.tile([128, 136], bf16, tag="stg")
                nc.scalar.activation(out=stg[:rc, 0:128], in_=h_ps[:rc, :],
                                     func=mybir.ActivationFunctionType.Copy)
                nc.vector.tensor_copy(out=stg[:rc, 128:136].bitcast(f32),
                                      in_=es_ps[:rc, :])
                nc.sync.dma_start(out=tab[d][1 + r0:1 + r0 + rc, 0:136],
                                  in_=stg[:rc, 0:136])

        # ---- phase 1b: per-core permuted e_dst tables ----
        for d in range(2 if PHASES >= 2 else 0):
            for t in range(NW):
                r0 = t * 128
                xt = sb.tile([128, F_IN], f32, tag="xt")
                nc.sync.dma_start(out=xt[:], in_=xp[d][r0:r0 + 128, :])
                xT_ps = ps_big.tile([128, 128], f32, tag="psb")
                nc.tensor.transpose(out=xT_ps[:], in_=xt[:], identity=id_f32[:])
                xT = sb.tile([128, 128], f32, tag="xT")
                nc.vector.tensor_copy(out=xT[:], in_=xT_ps[:])
                ed_ps = ps_sm.tile([128, H], f32, tag="pss")
                nc.tensor.matmul(out=ed_ps[:], lhsT=xT[:], rhs=wa_sb[d][:, H:2 * H],
                                 start=True, stop=True)
                ed_sb = sb.tile([128, H], f32, tag="ed_sb")
                nc.vector.tensor_copy(out=ed_sb[:], in_=ed_ps[:])
                nc.sync.dma_start(out=edp[d][r0:r0 + 128, :], in_=ed_sb[:])

        tc.strict_bb_all_engine_barrier()

        # ---- phase 2: gather + attention + aggregate + scatter ----
        qrot = 0
        for d in range(2 if PHASES >= 3 else 0):
            gc0 = 0  # running column offset into gidx[d]
            for w in range(NW):
                KL, KH = klo[d][w], khi[d][w]
                nblk = KL + KH
                if nblk == 0:
                    continue
                ncols = nblk * 8

                ed_t = sb_g.tile([128, H], f32, tag="ed_t")
                nc.sync.dma_start(out=ed_t[:], in_=edp[d][w * 128:(w + 1) * 128, :])
                it = sb_g.tile([128, NBMAX * 8], i16, tag="it")
                nc.sync.dma_start(out=it[:, :ncols], in_=gidx[d][:, gc0:gc0 + ncols])

                gt = sb_g.tile([128, NBMAX, TCOLS], bf16, tag="gt")
                lo_end = min(32768, TROWS)
                hi_end = min(HI_BASE + 32768, TROWS)
                for s0 in range(0, KL, MAX_CHUNK_SLOTS):
                    s1 = min(s0 + MAX_CHUNK_SLOTS, KL)
                    nc.gpsimd.dma_gather(
                        gt[:, s0:s1, :], tab[d][0:lo_end, :],
                        it[:, s0 * 8:s1 * 8], (s1 - s0) * 128,
                        nreg((s1 - s0) * 128), TCOLS, queue_num=0)
                    qrot += 1
                for s0 in range(0, KH, MAX_CHUNK_SLOTS):
                    s1 = min(s0 + MAX_CHUNK_SLOTS, KH)
                    nc.gpsimd.dma_gather(
                        gt[:, KL + s0:KL + s1, :], tab[d][HI_BASE:hi_end, :],
                        it[:, (KL + s0) * 8:(KL + s1) * 8], (s1 - s0) * 128,
                        nreg((s1 - s0) * 128), TCOLS, queue_num=0)
                    qrot += 1
                gc0 += ncols

                es_v = gt[:, :nblk, 128:136].bitcast(f32)  # [128, nblk, 4]
                pre = sb_g.tile([128, NBMAX, H], f32, tag="pre")
                nc.vector.tensor_tensor(
                    out=pre[:, :nblk, :], in0=es_v,
                    in1=ed_t[:].unsqueeze(1).to_broadcast([128, nblk, H]),
                    op=mybir.AluOpType.add)
                lr = sb_g.tile([128, NBMAX, H], f32, tag="lr")
                nc.vector.tensor_scalar(out=lr[:, :nblk, :], in0=pre[:, :nblk, :],
                                        scalar1=SLOPE, scalar2=None,
                                        op0=mybir.AluOpType.mult)
                nc.vector.tensor_tensor(out=pre[:, :nblk, :], in0=pre[:, :nblk, :],
                                        in1=lr[:, :nblk, :],
                                        op=mybir.AluOpType.max)
                wt = sb_g.tile([128, NBMAX, H], bf16, tag="wt")
                nc.scalar.activation(out=wt[:, :nblk, :], in_=pre[:, :nblk, :],
                                     func=mybir.ActivationFunctionType.Exp)

                ht = sb_g.tile([128, NBMAX, FEAT + H], bf16, tag="ht")
                nc.vector.tensor_tensor(
                    out=ht[:, :nblk, 0:FEAT].rearrange("p b (h c) -> p b h c", h=H),
                    in0=gt[:, :nblk, 0:FEAT].rearrange("p b (h c) -> p b h c", h=H),
                    in1=wt[:, :nblk, :].unsqueeze(3).to_broadcast(
                        [128, nblk, H, C_OUT]),
                    op=mybir.AluOpType.mult)
                nc.vector.tensor_scalar(
                    out=ht[:, :nblk, FEAT:FEAT + H], in0=wt[:, :nblk, :],
                    scalar1=2.0, scalar2=None, op0=mybir.AluOpType.mult)

                acc_ps = ps_acc.tile([128, FEAT + H], f32, tag="psa")
                for s in range(nblk):
                    nc.tensor.matmul(out=acc_ps[:], lhsT=id_bf[:],
                                     rhs=ht[:, s, :], start=(s == 0),
                                     stop=(s == nblk - 1))

                rec = sb_g.tile([128, H], f32, tag="rec")
                nc.vector.reciprocal(out=rec[:], in_=acc_ps[:, FEAT:FEAT + H])
                stg2 = sb_g.tile([128, FEAT], f32, tag="stg2")
                nc.vector.tensor_tensor(
                    out=stg2[:].rearrange("p (h c) -> p h c", h=H),
                    in0=acc_ps[:, 0:FEAT].rearrange("p (h c) -> p h c", h=H),
                    in1=rec[:].unsqueeze(2).to_broadcast([128, H, C_OUT]),
                    op=mybir.AluOpType.mult)

                st = sb_g.tile([128, 8], i16, tag="st")
                nc.sync.dma_start(out=st[:], in_=sidx[d][:, w * 8:(w + 1) * 8])
                nc.gpsimd.dma_scatter_add(
                    acc[d][:, :], stg2[:].unsqueeze(1), st[:], 128, nreg(128),
                    FEAT, queue_num=0)

        tc.strict_bb_all_engine_barrier()

        # ---- phase 3: blend + bias ----
        for t in range(NW if PHASES >= 4 else 0):
            r0 = t * 128
            rc = min(128, NPC - r0)
            af = sb.tile([128, FEAT], f32, tag="af")
            nc.sync.dma_start(out=af[:], in_=acc[0][r0:r0 + 128, :])
            ab = sb.tile([128, FEAT], f32, tag="ab")
            nc.sync.dma_start(out=ab[:], in_=acc[1][r0:r0 + 128, :])
            nc.vector.tensor_add(out=af[:], in0=af[:], in1=ab[:])
            nc.vector.tensor_add(out=af[:], in0=af[:], in1=bbar[:])
            nc.sync.dma_start(out=out_ext[r0:r0 + rc, :], in_=af[:rc, :])
        if PHASES < 4:
            for t in range(NW):
                r0 = t * 128
                rc = min(128, NPC - r0)
                if rc > 0:
                    nc.sync.dma_start(out=out_ext[r0:r0 + rc, :], in_=zt[:rc, :])

    return nc


# ---------------------------------------------------------------------------
# Walrus workaround: this build caps sync waits per instruction at 1; move
# extras onto same-engine NoOps inserted just before the owner.
# ---------------------------------------------------------------------------

def fix_swdge_queues(nc, nqueues=4):
    """Align each SWDGE instruction's queue_num with its Tile-assigned DMASW
    sem lane (queue = lane % nqueues) so per-lane sem counting stays ordered
    while descriptor generation spreads over the Q7 core pairs."""
    import re as _re
    names = {}
    try:
        names = dict(nc.m.ant_sem_names or {})
    except Exception:
        pass

    def lane_of(inst):
        si = inst.sync_info
        if not si or not si.on_update:
            return None
        for u in si.on_update:
            nm = getattr(u, "ant_name", None) or names.get(getattr(u, "id", -1), "")
            m = _re.match(r"DMASW(\d+)", nm or "")
            if m:
                return int(m.group(1))
        return None

    for f in nc.m.functions:
        for bb in f.blocks:
            for inst in bb.instructions:
                if type(inst).__name__ in ("InstDMAGatherAnt", "InstDMAScatterAddAnt"):
                    lane = lane_of(inst)
                    if lane is not None:
                        inst.queue_num = lane % nqueues
    return nc


def split_waits(nc):
    import concourse.mybir as mybir
    ctr = 0
    for f in nc.m.functions:
        for bb in f.blocks:
            out = []
            changed = False
            for inst in bb.instructions:
                si = inst.sync_info
                if si is not None and si.on_wait and len(si.on_wait) > 1:
                    waits = list(si.on_wait)
                    for w in waits[:-1]:
                        nop = mybir.InstNoOp(name=f"Wsplit-{ctr}", ins=[], outs=[])
                        ctr += 1
                        nop.engine = inst.engine
                        nop.sync_info = mybir.SyncInfo(on_wait=[w], on_update=[])
                        out.append(nop)
                    si.on_wait = waits[-1:]
                    inst.sync_info = si
                    changed = True
                out.append(inst)
            if changed:
                bb.instructions = out
    return nc


# ---------------------------------------------------------------------------
# Execution via PJRT (axon) — jit once, reuse across calls
# ---------------------------------------------------------------------------

_RUNNER_CACHE = {}


def _make_runner(nc, n_cores):
    import jax
    import numpy as _np
    import concourse.mybir as mybir
    from concourse.bass2jax import (
        _bass_exec_p, install_neuronx_cc_hook, partition_id_tensor)
    from jax.sharding import Mesh, PartitionSpec
    from jax.experimental.shard_map import shard_map

    install_neuronx_cc_hook()
    partition_name = nc.partition_id_tensor.name if nc.partition_id_tensor else None
    in_names, out_names, out_avals, zero_shapes = [], [], [], []
    for alloc in nc.m.functions[0].allocations:
        if not isinstance(alloc, mybir.MemoryLocationSet):
            continue
        name = alloc.memorylocations[0].name
        if alloc.kind == "ExternalInput":
            if name != partition_name:
                in_names.append(name)
        elif alloc.kind == "ExternalOutput":
            out_names.append(name)
            shape = tuple(alloc.tensor_shape)
            dtype = mybir.dt.np(alloc.dtype)
            out_avals.append(jax.core.ShapedArray(shape, dtype))
            zero_shapes.append((shape, dtype))
    n_params = len(in_names)
    n_outs = len(out_avals)
    all_in_names = list(in_names) + list(out_names)
    if partition_name is not None:
        all_in_names.append(partition_name)

    def _body(*args):
        operands = list(args)
        if partition_name is not None:
            operands.append(partition_id_tensor())
        outs = _bass_exec_p.bind(
            *operands,
            out_avals=tuple(out_avals),
            in_names=tuple(all_in_names),
            out_names=tuple(out_names),
            lowering_input_output_aliases=(),
            sim_require_finite=False,
            sim_require_nnan=False,
            nc=nc,
        )
        return tuple(outs)

    devices = jax.devices()[:n_cores]
    mesh = Mesh(_np.asarray(devices), ("core",))
    fn = jax.jit(
        shard_map(_body, mesh=mesh,
                  in_specs=(PartitionSpec("core"),) * (n_params + n_outs),
                  out_specs=(PartitionSpec("core"),) * n_outs,
                  check_rep=False),
        keep_unused=True,
    )

    def run(in_maps):
        per_core = [[_np.asarray(m[n]) for n in in_names] for m in in_maps]
        concat_in = [
            _np.concatenate([per_core[c][i] for c in range(n_cores)], axis=0)
            for i in range(n_params)
        ]
        concat_zeros = [
            _np.zeros((n_cores * s[0], *s[1:]), d) for s, d in zero_shapes
        ]
        sharding = jax.sharding.NamedSharding(mesh, PartitionSpec("core"))
        args = [jax.device_put(a, sharding) for a in concat_in + concat_zeros]
        out = fn(*args)
        jax.block_until_ready(out)
        return [
            {
                n: _np.asarray(out[i]).reshape(n_cores, *out_avals[i].shape)[c]
                for i, n in enumerate(out_names)
            }
            for c in range(n_cores)
        ], fn, args

    return run


def _get_runner(struct):
    key = (tuple(map(tuple, struct["klo"])), tuple(map(tuple, struct["khi"])))
    if key not in _RUNNER_CACHE:
        import concourse.mybir as mybir
        nc = build_program(struct)
        fix_swdge_queues(nc, nqueues=4)
        mybir.codegen_inst_isa_subclasses(nc)  # lower extended-ISA insts
        split_waits(nc)
        _RUNNER_CACHE.clear()
        _RUNNER_CACHE[key] = _make_runner(nc, NCORES)
    return _RUNNER_CACHE[key]


def kernel(x, edge_index, W1, a_src1, a_dst1, b1, W2, a_src2, a_dst2, b2):
    x = np.asarray(x, dtype=np.float32)
    in_maps, struct = host_prep(x, edge_index, a_src1, a_dst1,
                                a_src2, a_dst2, b1, b2)
    W1 = np.asarray(W1, dtype=np.float32)
    W2 = np.asarray(W2, dtype=np.float32)
    for m in in_maps:
        m["W1"] = W1
        m["W2"] = W2
    run = _get_runner(struct)
    results, _, _ = run(in_maps)
    out = np.concatenate([r["out"] for r in results], axis=0)
    return out.astype(np.float32)

